# revision 15
# baseline (speedup 1.0000x reference)
"""GroupInfoNCE loss kernel for 8 Trainium2 NeuronCores.

Strategy (row-sharded, AllGather-based, host-prepped, minimal I/O):
  - Host normalizes f1/f2 in f32, pre-scales x16 and quantizes to fp8e4m3,
    transposes to [D, rows]. Core k receives ONLY its own shards (0.25 MB
    each) — total kernel input ~0.5 MB per core (loss rel err ~3e-5).
  - On device, f2 shards are AllGather'd (DRAM->DRAM collective) into the
    full [2048, 1024] = 8 x [256, 1024] gathered f2T; the big GEMM streams
    against the gathered copy in canonical global column order.
  - The positive (diagonal) 1024x1024 block of S is recomputed from the
    LOCAL f2 shard, so positive-block extraction is core-independent and
    overlaps with the AllGather.
  - The 8192x8192 logits matrix never touches HBM: each [128,1024] GEMM
    tile is consumed in PSUM by ScalarE exp (scale=s immediate) -> bf16.
  - Row stats: VectorE 3D-AP reduce -> 16-wide block sums; all Ln ops are
    pinned after the Exp stream via a late-data bias operand so the
    scheduler never interleaves activation-table reloads (2 loads total).
  - Column stats: TensorE ones-matmul -> complete per-core column block
    sums craw [64, 8192]; reduced on device to a_part/b_part [1, 8192]
    (sum and sum-of-logs over the core's 64 groups). Host does the tiny
    O(GN) combine.
"""

import os
import numpy as np

GN, D = 8192, 256
NGRP = 16               # group length N
EPS = 0.1               # label smoothing
G = GN // NGRP          # 512 groups
NCORES = 8
RPC = GN // NCORES      # 1024 rows per core
NSTRIP = RPC // 128     # 8 strips of 128 rows
NJB = GN // 1024        # 8 j-blocks of 1024 columns

_cache = {}
last_results = None


def _build_program(scale: float):
    PRE = 16.0  # host fp8 pre-scale; folded out of the exp activation scale
    from contextlib import ExitStack
    import concourse.bass as bass  # noqa: F401
    import concourse.mybir as mybir
    import concourse.tile as tile
    from concourse import bacc

    f32 = mybir.dt.float32
    bf16 = mybir.dt.bfloat16
    f8 = mybir.dt.float8e4
    AF = mybir.ActivationFunctionType
    AX = mybir.AxisListType

    nc = bacc.Bacc(
        "TRN2",
        target_bir_lowering=False,
        debug=False,
        enable_asserts=False,
        num_devices=NCORES,
    )

    f1T_d = nc.dram_tensor("f1T", [D, RPC], f8, kind="ExternalInput").ap()
    f2Ts_d = nc.dram_tensor("f2Ts", [D, RPC], f8, kind="ExternalInput").ap()
    mask128_d = nc.dram_tensor("mask128", [128, 8], f32, kind="ExternalInput").ap()

    o_asum_d = nc.dram_tensor("o_asum", [128, NSTRIP], f32, kind="ExternalOutput").ap()
    o_slog_d = nc.dram_tensor("o_slog", [128, NSTRIP], f32, kind="ExternalOutput").ap()
    o_pos_d = nc.dram_tensor("o_pos", [128, NSTRIP], f32, kind="ExternalOutput").ap()
    o_pos2_d = nc.dram_tensor("o_pos2", [1, RPC], f32, kind="ExternalOutput").ap()
    o_ca_d = nc.dram_tensor("o_ca", [1, GN], f32, kind="ExternalOutput").ap()
    o_cb_d = nc.dram_tensor("o_cb", [1, GN], f32, kind="ExternalOutput").ap()

    with tile.TileContext(nc) as tc, ExitStack() as ctx:
        singles = ctx.enter_context(tc.tile_pool(name="singles", bufs=1))
        expp = ctx.enter_context(tc.tile_pool(name="expp", bufs=4))
        crawp = ctx.enter_context(tc.tile_pool(name="crawp", bufs=2))
        scratch = ctx.enter_context(tc.tile_pool(name="scratch", bufs=2))
        dram = ctx.enter_context(tc.tile_pool(name="dram", bufs=1, space="DRAM"))

        mask128_sb = singles.tile([128, 8], f32, name="mask128_sb")
        nc.sync.dma_start(out=mask128_sb, in_=mask128_d)

        # -------- constants built on device (no input bytes) --------
        # ones64[p, t, c] = 1 iff c == 8t + p//16  (colsum weights per strip)
        ones64_sb = singles.tile([128, NSTRIP, 64], bf16, name="ones64_sb")
        nc.vector.memset(ones64_sb, 0.0)
        for t in range(NSTRIP):
            nc.vector.tensor_copy(
                ones64_sb[:, t, 8 * t : 8 * t + 8], mask128_sb
            )
        # maskbd[p, j] = 1 iff p//16 == j//16 (block-diagonal 16x16 of ones)
        ones16 = singles.tile([128, 16], bf16, name="ones16")
        nc.vector.memset(ones16, 1.0)
        maskbd_sb = singles.tile([128, 128], bf16, name="maskbd_sb")
        for g in range(8):
            nc.vector.tensor_scalar_mul(
                maskbd_sb[:, 16 * g : 16 * g + 16],
                ones16,
                mask128_sb[:, g : g + 1],
            )
        ones128_sb = singles.tile([128, 1], bf16, name="ones128_sb")
        nc.vector.memset(ones128_sb, 1.0)
        ones64x1_sb = singles.tile([64, 1], f32, name="ones64x1_sb")
        nc.vector.memset(ones64x1_sb, 1.0)

        # -------- feature loads + AllGather of f2 shards --------
        f1T = [singles.tile([128, RPC], f8, name=f"f1T{h}") for h in (0, 1)]
        f2l = [singles.tile([128, RPC], f8, name=f"f2l{h}") for h in (0, 1)]
        for kc in (0, 1):
            nc.sync.dma_start(out=f1T[kc], in_=f1T_d[kc * 128 : (kc + 1) * 128, :])

        ag_in = dram.tile([D, RPC], f8, name="ag_in")
        ag_out = dram.tile([D * NCORES, RPC], f8, name="ag_out")
        nc.gpsimd.dma_start(ag_in[:], f2Ts_d)
        # local-shard SBUF copy reads the internal bounce, so the external
        # f2Ts tensor is touched exactly once
        for kc in (0, 1):
            nc.scalar.dma_start(
                out=f2l[kc], in_=ag_in[kc * 128 : (kc + 1) * 128, :]
            )
        nc.gpsimd.collective_compute(
            "AllGather",
            mybir.AluOpType.bypass,
            replica_groups=[list(range(NCORES))],
            ins=[ag_in.opt()],
            outs=[ag_out.opt()],
        )
        # gathered chunk jb = core jb's f2T shard: global cols [1024*jb, ...)
        f2g = [
            [
                singles.tile([128, 1024], f8, name=f"f2g{kc}_{jb}")
                for jb in range(NJB)
            ]
            for kc in (0, 1)
        ]
        for jb in range(NJB):
            for kc in (0, 1):
                eng = nc.sync if kc == 0 else nc.scalar
                eng.dma_start(
                    out=f2g[kc][jb],
                    in_=ag_out[D * jb + kc * 128 : D * jb + (kc + 1) * 128, :],
                )

        rowblk = [
            singles.tile([128, G], f32, name=f"rowblk{t}", tag=f"rowblk{t}")
            for t in range(NSTRIP)
        ]
        o_asum_sb = singles.tile([128, NSTRIP], f32, name="o_asum_sb")
        o_slog_sb = singles.tile([128, NSTRIP], f32, name="o_slog_sb")
        o_pos_sb = singles.tile([128, NSTRIP], f32, name="o_pos_sb")
        o_pos2_sb = singles.tile([1, RPC], f32, name="o_pos2_sb")
        o_ca_sb = singles.tile([1, GN], f32, name="o_ca_sb")
        o_cb_sb = singles.tile([1, GN], f32, name="o_cb_sb")
        posv = singles.tile([128, NSTRIP], f32, name="posv")
        craw_all = [
            singles.tile([64, 1024], f32, name=f"craw{jb}", tag=f"craw{jb}")
            for jb in range(NJB)
        ]

        # -------- main pools; diag path shares them ----------------------
        with tc.tile_pool(name="psg", bufs=2, space="PSUM") as psg, \
             tc.tile_pool(name="psc", bufs=2, space="PSUM") as psc:
            # ---- diagonal-block path (local shard; overlaps AllGather) ----
            # All 8 diag strips go into one [128, 1024] PSUM tile: strip t's
            # [128, 128] diag GEMM lands at columns [128t, 128t+128).
            dps = psg.tile([128, 1024], f32, tag="gemm", name="dps")
            for t in range(NSTRIP):
                for kc in (0, 1):
                    nc.tensor.matmul(
                        dps[:, t * 128 : (t + 1) * 128],
                        lhsT=f1T[kc][:, t * 128 : (t + 1) * 128],
                        rhs=f2l[kc][:, t * 128 : (t + 1) * 128],
                        start=(kc == 0),
                        stop=(kc == 1),
                    )
            dmask = scratch.tile([128, 1024], bf16, tag="dmask", name="dmask")
            nc.scalar.activation(dmask, dps, AF.Exp, scale=scale / (PRE * PRE))
            for t in range(NSTRIP):
                nc.vector.tensor_mul(
                    dmask[:, t * 128 : (t + 1) * 128],
                    dmask[:, t * 128 : (t + 1) * 128],
                    maskbd_sb,
                )
            # row side: 16-wide block sums; strip t's own group is column
            # 8t + p//16 of posblk_all
            posblk_all = scratch.tile([128, 64], f32, tag="posblk", name="posblk")
            nc.vector.reduce_sum(
                out=posblk_all,
                in_=dmask.rearrange("p (g n) -> p g n", n=NGRP),
                axis=AX.X,
            )
            for t in range(NSTRIP):
                pose = scratch.tile([128, 8], f32, tag="pose", name="pose")
                nc.vector.tensor_mul(
                    pose, posblk_all[:, 8 * t : 8 * t + 8], mask128_sb
                )
                nc.vector.reduce_sum(
                    out=posv[:, t : t + 1], in_=pose, axis=AX.X
                )
            # col side: colsums of the masked tile = positive block sums
            pcps = psc.tile([64, 1024], f32, tag="colps", name="pcps")
            for h in (0, 1):
                nc.tensor.matmul(
                    pcps[0:1, h * 512 : (h + 1) * 512],
                    lhsT=ones128_sb,
                    rhs=dmask[:, h * 512 : (h + 1) * 512],
                    start=True,
                    stop=True,
                )
            pc_sb = singles.tile([1, RPC], f32, name="pc_sb")
            nc.vector.tensor_copy(pc_sb, pcps[0:1, :])

            # -------- main fused GEMM + stats loop -----------------------
            for jb in range(NJB):
                colps = psc.tile([64, 1024], f32, tag="colps", name="colps")
                for t in range(NSTRIP):
                    ps = psg.tile([128, 1024], f32, tag="gemm", name="ps")
                    for kc in (0, 1):
                        for h in (0, 1):
                            nc.tensor.matmul(
                                ps[:, h * 512 : (h + 1) * 512],
                                lhsT=f1T[kc][:, t * 128 : (t + 1) * 128],
                                rhs=f2g[kc][jb][:, h * 512 : (h + 1) * 512],
                                start=(kc == 0),
                                stop=(kc == 1),
                            )
                    expb = expp.tile([128, 1024], bf16, tag="exp", name="expb")
                    nc.scalar.activation(expb, ps, AF.Exp, scale=scale / (PRE * PRE))
                    nc.vector.reduce_sum(
                        out=rowblk[t][:, jb * 64 : (jb + 1) * 64],
                        in_=expb.rearrange("p (g n) -> p g n", n=NGRP),
                        axis=AX.X,
                    )
                    for h in (0, 1):
                        nc.tensor.matmul(
                            colps[:, h * 512 : (h + 1) * 512],
                            lhsT=ones64_sb[:, t, :],
                            rhs=expb[:, h * 512 : (h + 1) * 512],
                            start=(t == 0),
                            stop=(t == NSTRIP - 1),
                        )
                craw_sb = craw_all[jb]
                nc.vector.tensor_copy(craw_sb, colps)

            # -------- deferred log-domain tails (single Exp->Ln table swap) ----
            for t in range(NSTRIP):
                nc.vector.reduce_sum(
                    out=o_asum_sb[:, t : t + 1], in_=rowblk[t], axis=AX.X
                )
            # late_zero is data-dependent on the last row reduction, which
            # pins the early-ready Ln ops below AFTER the main Exp stream so
            # the scheduler cannot interleave activation-table reloads with it
            late_zero = singles.tile([128, 1], f32, name="late_zero")
            nc.vector.tensor_scalar_mul(
                late_zero, o_asum_sb[:, NSTRIP - 1 : NSTRIP], 0.0
            )
            nc.scalar.activation(o_pos_sb, posv, AF.Ln, bias=late_zero)
            nc.scalar.activation(
                o_pos2_sb, pc_sb, AF.Ln, bias=late_zero[0:1, :]
            )
            for t in range(NSTRIP):
                nc.scalar.activation(
                    rowblk[t], rowblk[t], AF.Ln, bias=late_zero,
                    accum_out=o_slog_sb[:, t : t + 1],
                )
            for jb in range(NJB):
                craw_sb = craw_all[jb]
                blog_sb = crawp.tile([64, 1024], f32, tag="blog_sb", name="blog_sb")
                nc.scalar.activation(
                    blog_sb, craw_sb, AF.Ln, bias=late_zero[0:64, :]
                )
                # partition-reduce (64 groups -> 1) via ones matmul, into a
                # colps-tagged tile's rows to stay inside the psc pool
                red = psc.tile([64, 1024], f32, tag="colps", name="red")
                for h in (0, 1):
                    nc.tensor.matmul(
                        red[0:1, h * 512 : (h + 1) * 512],
                        lhsT=ones64x1_sb,
                        rhs=craw_sb[:, h * 512 : (h + 1) * 512],
                        start=True,
                        stop=True,
                    )
                    nc.tensor.matmul(
                        red[32:33, h * 512 : (h + 1) * 512],
                        lhsT=ones64x1_sb,
                        rhs=blog_sb[:, h * 512 : (h + 1) * 512],
                        start=True,
                        stop=True,
                    )
                nc.vector.tensor_copy(
                    o_ca_sb[:, jb * 1024 : (jb + 1) * 1024], red[0:1, :]
                )
                nc.vector.tensor_copy(
                    o_cb_sb[:, jb * 1024 : (jb + 1) * 1024], red[32:33, :]
                )

        nc.sync.dma_start(out=o_asum_d, in_=o_asum_sb)
        nc.sync.dma_start(out=o_slog_d, in_=o_slog_sb)
        nc.sync.dma_start(out=o_pos_d, in_=o_pos_sb)
        nc.sync.dma_start(out=o_pos2_d, in_=o_pos2_sb)
        nc.sync.dma_start(out=o_ca_d, in_=o_ca_sb)
        nc.sync.dma_start(out=o_cb_d, in_=o_cb_sb)

    nc.compile()
    return nc


def _constants():
    p = np.arange(128)
    mask128 = np.zeros((128, 8), dtype=np.float32)
    mask128[p, p // 16] = 1.0
    return mask128


def build_in_maps(image_features1, image_features2, logit_scale):
    """Host prep: normalize, cast bf16, transpose, per-core shard."""
    import ml_dtypes

    f1 = np.asarray(image_features1, dtype=np.float32)
    f2 = np.asarray(image_features2, dtype=np.float32)
    s = float(np.asarray(logit_scale).reshape(-1)[0])

    f1n = f1 / np.linalg.norm(f1, axis=-1, keepdims=True)
    f2n = f2 / np.linalg.norm(f2, axis=-1, keepdims=True)
    PRE = 16.0
    f1nT = np.ascontiguousarray((f1n.T * PRE).astype(ml_dtypes.float8_e4m3))
    f2nT = np.ascontiguousarray((f2n.T * PRE).astype(ml_dtypes.float8_e4m3))

    mask128 = _constants()
    in_maps = []
    for k in range(NCORES):
        in_maps.append(
            {
                "f1T": np.ascontiguousarray(f1nT[:, k * RPC : (k + 1) * RPC]),
                "f2Ts": np.ascontiguousarray(f2nT[:, k * RPC : (k + 1) * RPC]),
                "mask128": mask128,
            }
        )
    return s, in_maps


def combine_host(results):
    """O(GN) host combine of per-core row/column stats -> scalar loss."""
    eps = EPS
    S1 = 0.0
    for k in range(NCORES):
        r = results[k]
        asum = r["o_asum"].astype(np.float64)  # [128, NSTRIP] sum_j exp
        slog = r["o_slog"].astype(np.float64)  # [128, NSTRIP] sum_g log blocksum
        pos = r["o_pos"].astype(np.float64)  # [128, NSTRIP] log blocksum at pos
        per_row = np.log(asum) - (1.0 - eps) * pos - (eps / G) * slog
        S1 += per_row.sum()

    a_tot = np.zeros(GN, dtype=np.float64)
    b_tot = np.zeros(GN, dtype=np.float64)
    lpos2 = np.zeros(GN, dtype=np.float64)
    for k in range(NCORES):
        a_tot += results[k]["o_ca"].astype(np.float64).ravel()
        b_tot += results[k]["o_cb"].astype(np.float64).ravel()
        lpos2[k * RPC : (k + 1) * RPC] = (
            results[k]["o_pos2"].astype(np.float64).ravel()
        )
    per_row2 = np.log(a_tot) - (1.0 - eps) * lpos2 - (eps / G) * b_tot
    S2 = per_row2.sum()

    return (S1 + S2) / (2.0 * GN)


def kernel(image_features1, image_features2, logit_scale):
    global last_results
    from concourse.bass_utils import run_bass_kernel_spmd

    s, in_maps = build_in_maps(image_features1, image_features2, logit_scale)

    key = round(s, 9)
    if key not in _cache:
        _cache[key] = _build_program(s)
    nc = _cache[key]

    try:
        res = run_bass_kernel_spmd(
            nc,
            in_maps,
            core_ids=list(range(NCORES)),
            trace=bool(os.environ.get("KTRACE")),
        )
    except ModuleNotFoundError:
        # axon build without NTFF profiling hooks — rerun without trace
        res = run_bass_kernel_spmd(
            nc, in_maps, core_ids=list(range(NCORES)), trace=False
        )
    last_results = res

    loss = combine_host(res.results)
    return np.array(loss, dtype=np.float32)


# revision 17
# speedup vs baseline: 2.2055x; 2.2055x over previous
"""GroupInfoNCE loss kernel for 8 Trainium2 NeuronCores.

Strategy (row-sharded, AllGather-based, host-prepped, minimal I/O):
  - Host normalizes f1/f2 in f32, pre-scales x16 and quantizes to fp8e4m3,
    transposes to [D, rows]. Core k receives ONLY its own shards (0.25 MB
    each) — total kernel input ~0.5 MB per core (loss rel err ~3e-5).
  - On device, f2 shards are AllGather'd (DRAM->DRAM collective) into the
    full [2048, 1024] = 8 x [256, 1024] gathered f2T; the big GEMM streams
    against the gathered copy in canonical global column order.
  - The positive (diagonal) 1024x1024 block of S is recomputed from the
    LOCAL f2 shard, so positive-block extraction is core-independent and
    overlaps with the AllGather.
  - The 8192x8192 logits matrix never touches HBM: each [128,1024] GEMM
    tile is consumed in PSUM by ScalarE exp (scale=s immediate) -> bf16.
  - Row stats: VectorE 3D-AP reduce -> 16-wide block sums; all Ln ops are
    pinned after the Exp stream via a late-data bias operand so the
    scheduler never interleaves activation-table reloads (2 loads total).
  - Column stats: TensorE ones-matmul -> complete per-core column block
    sums craw [64, 8192]; reduced on device to a_part/b_part [1, 8192]
    (sum and sum-of-logs over the core's 64 groups). Host does the tiny
    O(GN) combine.
"""

import os
import numpy as np

GN, D = 8192, 256
NGRP = 16               # group length N
EPS = 0.1               # label smoothing
G = GN // NGRP          # 512 groups
NCORES = 8
RPC = GN // NCORES      # 1024 rows per core
NSTRIP = RPC // 128     # 8 strips of 128 rows
NJB = GN // 1024        # 8 j-blocks of 1024 columns

_cache = {}
last_results = None


def _build_program(scale: float):
    PRE = 16.0  # host fp8 pre-scale; folded out of the exp activation scale
    from contextlib import ExitStack
    import concourse.bass as bass  # noqa: F401
    import concourse.mybir as mybir
    import concourse.tile as tile
    from concourse import bacc

    f32 = mybir.dt.float32
    bf16 = mybir.dt.bfloat16
    f8 = mybir.dt.float8e4
    AF = mybir.ActivationFunctionType
    AX = mybir.AxisListType
    ALU = mybir.AluOpType

    nc = bacc.Bacc(
        "TRN2",
        target_bir_lowering=False,
        debug=False,
        enable_asserts=False,
        num_devices=NCORES,
    )

    f1T_d = nc.dram_tensor("f1T", [D, RPC], f8, kind="ExternalInput").ap()
    f2Ts_d = nc.dram_tensor("f2Ts", [D, RPC], f8, kind="ExternalInput").ap()

    o_asum_d = nc.dram_tensor("o_asum", [128, NSTRIP], f32, kind="ExternalOutput").ap()
    o_slog_d = nc.dram_tensor("o_slog", [128, NSTRIP], f32, kind="ExternalOutput").ap()
    o_pos_d = nc.dram_tensor("o_pos", [128, NSTRIP], f32, kind="ExternalOutput").ap()
    o_pos2_d = nc.dram_tensor("o_pos2", [1, RPC], f32, kind="ExternalOutput").ap()
    o_ca_d = nc.dram_tensor("o_ca", [1, GN], f32, kind="ExternalOutput").ap()
    o_cb_d = nc.dram_tensor("o_cb", [1, GN], f32, kind="ExternalOutput").ap()

    with tile.TileContext(nc) as tc, ExitStack() as ctx:
        singles = ctx.enter_context(tc.tile_pool(name="singles", bufs=1))
        expp = ctx.enter_context(tc.tile_pool(name="expp", bufs=4))
        crawp = ctx.enter_context(tc.tile_pool(name="crawp", bufs=2))
        scratch = ctx.enter_context(tc.tile_pool(name="scratch", bufs=2))
        dram = ctx.enter_context(tc.tile_pool(name="dram", bufs=1, space="DRAM"))

        # -------- constants built on device (no input bytes) --------
        # mask128[p, g] = 1 iff g == p//16, via two affine selects on the
        # iota value v(p, g) = p - 16g (keep 0 <= v <= 15)
        ones8 = singles.tile([128, 8], f32, name="ones8")
        nc.vector.memset(ones8, 1.0)
        mtmp = singles.tile([128, 8], f32, name="mtmp")
        nc.gpsimd.affine_select(
            mtmp, ones8, pattern=[[-16, 8]], compare_op=ALU.is_ge,
            fill=0.0, base=0, channel_multiplier=1,
        )
        mask128_sb = singles.tile([128, 8], f32, name="mask128_sb")
        nc.gpsimd.affine_select(
            mask128_sb, mtmp, pattern=[[16, 8]], compare_op=ALU.is_ge,
            fill=0.0, base=15, channel_multiplier=-1,
        )
        # ones64[p, t, c] = 1 iff c == 8t + p//16  (colsum weights per strip)
        ones64_sb = singles.tile([128, NSTRIP, 64], bf16, name="ones64_sb")
        nc.vector.memset(ones64_sb, 0.0)
        for t in range(NSTRIP):
            nc.vector.tensor_copy(
                ones64_sb[:, t, 8 * t : 8 * t + 8], mask128_sb
            )
        # maskbd[p, j] = 1 iff p//16 == j//16 (block-diagonal 16x16 of ones)
        ones16 = singles.tile([128, 16], bf16, name="ones16")
        nc.vector.memset(ones16, 1.0)
        maskbd_sb = singles.tile([128, 128], bf16, name="maskbd_sb")
        for g in range(8):
            nc.vector.tensor_scalar_mul(
                maskbd_sb[:, 16 * g : 16 * g + 16],
                ones16,
                mask128_sb[:, g : g + 1],
            )
        ones128_sb = singles.tile([128, 1], bf16, name="ones128_sb")
        nc.vector.memset(ones128_sb, 1.0)
        ones64x1_sb = singles.tile([64, 1], f32, name="ones64x1_sb")
        nc.vector.memset(ones64x1_sb, 1.0)

        # -------- feature loads + AllGather of f2 shards --------
        f1T = [singles.tile([128, RPC], f8, name=f"f1T{h}") for h in (0, 1)]
        f2l = [singles.tile([128, RPC], f8, name=f"f2l{h}") for h in (0, 1)]
        for kc in (0, 1):
            nc.sync.dma_start(out=f1T[kc], in_=f1T_d[kc * 128 : (kc + 1) * 128, :])

        ag_in = dram.tile([D, RPC], f8, name="ag_in")
        ag_out = dram.tile([D * NCORES, RPC], f8, name="ag_out")
        nc.gpsimd.dma_start(ag_in[:], f2Ts_d)
        # local-shard SBUF copy reads the internal bounce, so the external
        # f2Ts tensor is touched exactly once
        for kc in (0, 1):
            nc.scalar.dma_start(
                out=f2l[kc], in_=ag_in[kc * 128 : (kc + 1) * 128, :]
            )
        nc.gpsimd.collective_compute(
            "AllGather",
            mybir.AluOpType.bypass,
            replica_groups=[list(range(NCORES))],
            ins=[ag_in.opt()],
            outs=[ag_out.opt()],
        )
        # gathered chunk jb = core jb's f2T shard: global cols [1024*jb, ...)
        f2g = [
            [
                singles.tile([128, 1024], f8, name=f"f2g{kc}_{jb}")
                for jb in range(NJB)
            ]
            for kc in (0, 1)
        ]
        for jb in range(NJB):
            for kc in (0, 1):
                eng = nc.sync if kc == 0 else nc.scalar
                eng.dma_start(
                    out=f2g[kc][jb],
                    in_=ag_out[D * jb + kc * 128 : D * jb + (kc + 1) * 128, :],
                )

        rowblk = [
            singles.tile([128, G], f32, name=f"rowblk{t}", tag=f"rowblk{t}")
            for t in range(NSTRIP)
        ]
        o_asum_sb = singles.tile([128, NSTRIP], f32, name="o_asum_sb")
        o_slog_sb = singles.tile([128, NSTRIP], f32, name="o_slog_sb")
        o_pos_sb = singles.tile([128, NSTRIP], f32, name="o_pos_sb")
        o_pos2_sb = singles.tile([1, RPC], f32, name="o_pos2_sb")
        o_ca_sb = singles.tile([1, GN], f32, name="o_ca_sb")
        o_cb_sb = singles.tile([1, GN], f32, name="o_cb_sb")
        posv = singles.tile([128, NSTRIP], f32, name="posv")
        craw_all = [
            singles.tile([64, 1024], f32, name=f"craw{jb}", tag=f"craw{jb}")
            for jb in range(NJB)
        ]

        # -------- main pools; diag path shares them ----------------------
        with tc.tile_pool(name="psg", bufs=2, space="PSUM") as psg, \
             tc.tile_pool(name="psc", bufs=2, space="PSUM") as psc:
            # ---- diagonal-block path (local shard; overlaps AllGather) ----
            # All 8 diag strips go into one [128, 1024] PSUM tile: strip t's
            # [128, 128] diag GEMM lands at columns [128t, 128t+128).
            dps = psg.tile([128, 1024], f32, tag="gemm", name="dps")
            for t in range(NSTRIP):
                for kc in (0, 1):
                    nc.tensor.matmul(
                        dps[:, t * 128 : (t + 1) * 128],
                        lhsT=f1T[kc][:, t * 128 : (t + 1) * 128],
                        rhs=f2l[kc][:, t * 128 : (t + 1) * 128],
                        start=(kc == 0),
                        stop=(kc == 1),
                    )
            dmask = scratch.tile([128, 1024], bf16, tag="dmask", name="dmask")
            nc.scalar.activation(dmask, dps, AF.Exp, scale=scale / (PRE * PRE))
            for t in range(NSTRIP):
                nc.vector.tensor_mul(
                    dmask[:, t * 128 : (t + 1) * 128],
                    dmask[:, t * 128 : (t + 1) * 128],
                    maskbd_sb,
                )
            # row side: 16-wide block sums; strip t's own group is column
            # 8t + p//16 of posblk_all
            posblk_all = scratch.tile([128, 64], f32, tag="posblk", name="posblk")
            nc.vector.reduce_sum(
                out=posblk_all,
                in_=dmask.rearrange("p (g n) -> p g n", n=NGRP),
                axis=AX.X,
            )
            for t in range(NSTRIP):
                pose = scratch.tile([128, 8], f32, tag="pose", name="pose")
                nc.vector.tensor_mul(
                    pose, posblk_all[:, 8 * t : 8 * t + 8], mask128_sb
                )
                nc.vector.reduce_sum(
                    out=posv[:, t : t + 1], in_=pose, axis=AX.X
                )
            # col side: colsums of the masked tile = positive block sums
            pcps = psc.tile([64, 1024], f32, tag="colps", name="pcps")
            for h in (0, 1):
                nc.tensor.matmul(
                    pcps[0:1, h * 512 : (h + 1) * 512],
                    lhsT=ones128_sb,
                    rhs=dmask[:, h * 512 : (h + 1) * 512],
                    start=True,
                    stop=True,
                )
            pc_sb = singles.tile([1, RPC], f32, name="pc_sb")
            nc.vector.tensor_copy(pc_sb, pcps[0:1, :])

            # -------- main fused GEMM + stats loop -----------------------
            for jb in range(NJB):
                colps = psc.tile([64, 1024], f32, tag="colps", name="colps")
                for t in range(NSTRIP):
                    ps = psg.tile([128, 1024], f32, tag="gemm", name="ps")
                    for kc in (0, 1):
                        for h in (0, 1):
                            nc.tensor.matmul(
                                ps[:, h * 512 : (h + 1) * 512],
                                lhsT=f1T[kc][:, t * 128 : (t + 1) * 128],
                                rhs=f2g[kc][jb][:, h * 512 : (h + 1) * 512],
                                start=(kc == 0),
                                stop=(kc == 1),
                            )
                    expb = expp.tile([128, 1024], bf16, tag="exp", name="expb")
                    nc.scalar.activation(expb, ps, AF.Exp, scale=scale / (PRE * PRE))
                    nc.vector.reduce_sum(
                        out=rowblk[t][:, jb * 64 : (jb + 1) * 64],
                        in_=expb.rearrange("p (g n) -> p g n", n=NGRP),
                        axis=AX.X,
                    )
                    for h in (0, 1):
                        nc.tensor.matmul(
                            colps[:, h * 512 : (h + 1) * 512],
                            lhsT=ones64_sb[:, t, :],
                            rhs=expb[:, h * 512 : (h + 1) * 512],
                            start=(t == 0),
                            stop=(t == NSTRIP - 1),
                        )
                craw_sb = craw_all[jb]
                nc.vector.tensor_copy(craw_sb, colps)

            # -------- deferred log-domain tails (single Exp->Ln table swap) ----
            for t in range(NSTRIP):
                nc.vector.reduce_sum(
                    out=o_asum_sb[:, t : t + 1], in_=rowblk[t], axis=AX.X
                )
            # late_zero is data-dependent on the last row reduction, which
            # pins the early-ready Ln ops below AFTER the main Exp stream so
            # the scheduler cannot interleave activation-table reloads with it
            late_zero = singles.tile([128, 1], f32, name="late_zero")
            nc.vector.tensor_scalar_mul(
                late_zero, o_asum_sb[:, NSTRIP - 1 : NSTRIP], 0.0
            )
            nc.scalar.activation(o_pos_sb, posv, AF.Ln, bias=late_zero)
            nc.scalar.activation(
                o_pos2_sb, pc_sb, AF.Ln, bias=late_zero[0:1, :]
            )
            for t in range(NSTRIP):
                nc.scalar.activation(
                    rowblk[t], rowblk[t], AF.Ln, bias=late_zero,
                    accum_out=o_slog_sb[:, t : t + 1],
                )
            for jb in range(NJB):
                craw_sb = craw_all[jb]
                blog_sb = crawp.tile([64, 1024], f32, tag="blog_sb", name="blog_sb")
                nc.scalar.activation(
                    blog_sb, craw_sb, AF.Ln, bias=late_zero[0:64, :]
                )
                # partition-reduce (64 groups -> 1) via ones matmul, into a
                # colps-tagged tile's rows to stay inside the psc pool
                red = psc.tile([64, 1024], f32, tag="colps", name="red")
                for h in (0, 1):
                    nc.tensor.matmul(
                        red[0:1, h * 512 : (h + 1) * 512],
                        lhsT=ones64x1_sb,
                        rhs=craw_sb[:, h * 512 : (h + 1) * 512],
                        start=True,
                        stop=True,
                    )
                    nc.tensor.matmul(
                        red[32:33, h * 512 : (h + 1) * 512],
                        lhsT=ones64x1_sb,
                        rhs=blog_sb[:, h * 512 : (h + 1) * 512],
                        start=True,
                        stop=True,
                    )
                nc.vector.tensor_copy(
                    o_ca_sb[:, jb * 1024 : (jb + 1) * 1024], red[0:1, :]
                )
                nc.vector.tensor_copy(
                    o_cb_sb[:, jb * 1024 : (jb + 1) * 1024], red[32:33, :]
                )

        nc.sync.dma_start(out=o_asum_d, in_=o_asum_sb)
        nc.sync.dma_start(out=o_slog_d, in_=o_slog_sb)
        nc.sync.dma_start(out=o_pos_d, in_=o_pos_sb)
        nc.sync.dma_start(out=o_pos2_d, in_=o_pos2_sb)
        nc.sync.dma_start(out=o_ca_d, in_=o_ca_sb)
        nc.sync.dma_start(out=o_cb_d, in_=o_cb_sb)

    nc.compile()
    return nc


def build_in_maps(image_features1, image_features2, logit_scale):
    """Host prep: normalize, cast bf16, transpose, per-core shard."""
    import ml_dtypes

    f1 = np.asarray(image_features1, dtype=np.float32)
    f2 = np.asarray(image_features2, dtype=np.float32)
    s = float(np.asarray(logit_scale).reshape(-1)[0])

    f1n = f1 / np.linalg.norm(f1, axis=-1, keepdims=True)
    f2n = f2 / np.linalg.norm(f2, axis=-1, keepdims=True)
    PRE = 16.0
    f1nT = np.ascontiguousarray((f1n.T * PRE).astype(ml_dtypes.float8_e4m3))
    f2nT = np.ascontiguousarray((f2n.T * PRE).astype(ml_dtypes.float8_e4m3))

    in_maps = []
    for k in range(NCORES):
        in_maps.append(
            {
                "f1T": np.ascontiguousarray(f1nT[:, k * RPC : (k + 1) * RPC]),
                "f2Ts": np.ascontiguousarray(f2nT[:, k * RPC : (k + 1) * RPC]),
            }
        )
    return s, in_maps


def combine_host(results):
    """O(GN) host combine of per-core row/column stats -> scalar loss."""
    eps = EPS
    S1 = 0.0
    for k in range(NCORES):
        r = results[k]
        asum = r["o_asum"].astype(np.float64)  # [128, NSTRIP] sum_j exp
        slog = r["o_slog"].astype(np.float64)  # [128, NSTRIP] sum_g log blocksum
        pos = r["o_pos"].astype(np.float64)  # [128, NSTRIP] log blocksum at pos
        per_row = np.log(asum) - (1.0 - eps) * pos - (eps / G) * slog
        S1 += per_row.sum()

    a_tot = np.zeros(GN, dtype=np.float64)
    b_tot = np.zeros(GN, dtype=np.float64)
    lpos2 = np.zeros(GN, dtype=np.float64)
    for k in range(NCORES):
        a_tot += results[k]["o_ca"].astype(np.float64).ravel()
        b_tot += results[k]["o_cb"].astype(np.float64).ravel()
        lpos2[k * RPC : (k + 1) * RPC] = (
            results[k]["o_pos2"].astype(np.float64).ravel()
        )
    per_row2 = np.log(a_tot) - (1.0 - eps) * lpos2 - (eps / G) * b_tot
    S2 = per_row2.sum()

    return (S1 + S2) / (2.0 * GN)


def kernel(image_features1, image_features2, logit_scale):
    global last_results
    from concourse.bass_utils import run_bass_kernel_spmd

    s, in_maps = build_in_maps(image_features1, image_features2, logit_scale)

    key = round(s, 9)
    if key not in _cache:
        _cache[key] = _build_program(s)
    nc = _cache[key]

    try:
        res = run_bass_kernel_spmd(
            nc,
            in_maps,
            core_ids=list(range(NCORES)),
            trace=bool(os.environ.get("KTRACE")),
        )
    except ModuleNotFoundError:
        # axon build without NTFF profiling hooks — rerun without trace
        res = run_bass_kernel_spmd(
            nc, in_maps, core_ids=list(range(NCORES)), trace=False
        )
    last_results = res

    loss = combine_host(res.results)
    return np.array(loss, dtype=np.float32)
